# revision 1
# baseline (speedup 1.0000x reference)
"""Trainium2 Bass kernel for nn_LpAlignEntropyLoss.

Loss over three views z1,z2,z3 (each [8192,128] f32):
  for each pair (i<j):
    pos += mean_m ||zi_m - zj_m||
    neg += mean_m [ ln(sum_n exp(-d_mn)) - ln(B) ],  d = cdist(zi, zj)
  loss = (0.5*pos + 0.5*neg) / 3

Strategy: shard the 8192 rows across 8 cores (1024 each). Each core holds
all three views transposed ([128=D, 8192=B]) in SBUF as bf16, computes its
row-block of each pairwise squared-distance matrix with PE matmuls: the
-2*dot term is the main bf16 matmul, the +b2[n] column-norm term is folded
in as a K=1 accumulating matmul, and the +a2[m] row-norm term rides the
ACT bias. ScalarE then does sqrt (PSUM->SBUF fp16) and exp(16-d) with a
fused row-accumulate, batched per m-block by activation-table set to
bound table-switch cost. Host sums the 8 partial scalars; no collectives.
"""

import math

import numpy as np
import ml_dtypes

import concourse.bacc as bacc
import concourse.bass as bass
import concourse.mybir as mybir
import concourse.tile as tile
from concourse.tile import add_dep_helper
from concourse.bass_utils import run_bass_kernel_spmd

B, D = 8192, 128
NCORES = 8
ML = B // NCORES          # rows per core (1024)
MB = ML // 128            # m-blocks per core (8)
NCHUNK = 2048             # psum chunk (4 banks)
NQ = B // NCHUNK          # chunks per row (4)
PAIRS = [(0, 1), (0, 2), (1, 2)]
TAU = 1.0
ALPHA = 0.5
B2_CENTER = 128.0         # E[||z||^2] for z~N(0,I_128); centers the bf16 bias row
EXP_SHIFT = 16.0          # e^(SHIFT-d) keeps fp16 in range for d in [9, 27]

F32 = mybir.dt.float32
BF16 = mybir.dt.bfloat16
FP16 = mybir.dt.float16
AF = mybir.ActivationFunctionType
ALU = mybir.AluOpType
AX = mybir.AxisListType



def build(nc: bacc.Bacc):
    zt = [nc.dram_tensor(f"zt{v}", [D, B], BF16, kind="ExternalInput") for v in range(3)]
    blk = [nc.dram_tensor(f"blk{v}", [D, ML], BF16, kind="ExternalInput") for v in range(3)]
    b2h_in = {j: nc.dram_tensor(f"b2h{j}", [1, B], BF16, kind="ExternalInput")
              for j in sorted({j for _, j in PAIRS})}
    biasp_in = nc.dram_tensor("biaspall", [128, 3 * MB], F32, kind="ExternalInput")
    sqpos_in = nc.dram_tensor("sqposall", [128, 3 * MB], F32, kind="ExternalInput")
    out = nc.dram_tensor("out", [2, 1], F32, kind="ExternalOutput")

    rhs_views = sorted({j for _, j in PAIRS})  # views used as columns (1, 2)

    with tile.TileContext(nc) as tc:
        with tc.tile_pool(name="persist", bufs=1) as persist:
            # ---- persistent SBUF ----
            rhs_views_l = sorted({j for _, j in PAIRS})
            ztc = {j: [persist.tile([D, NCHUNK], BF16, tag=f"ztc{j}_{q}", name=f"ztc{j}_{q}")
                       for q in range(NQ)] for j in rhs_views_l}
            blks = [persist.tile([D, ML], BF16, tag=f"blks{v}", name=f"blks{v}") for v in range(3)]
            b2half = {j: persist.tile([1, B], BF16, tag=f"b2h{j}", name=f"b2h{j}") for j in rhs_views_l}
            biasp_t = persist.tile([128, 3 * MB], F32, tag="biaspall")
            sqpos = persist.tile([128, 3 * MB], F32, tag="sqposall")
            # small norm/bias inputs first: the first bias matmul gates on them
            for j in rhs_views_l:
                nc.sync.dma_start(b2half[j][:], b2h_in[j][:])
            nc.sync.dma_start(biasp_t[:], biasp_in[:])
            nc.sync.dma_start(sqpos[:], sqpos_in[:])
            nc.sync.dma_start(ztc[1][0][:], zt[1][:, 0:NCHUNK])
            nc.sync.dma_start(blks[0][:], blk[0][:])
            nc.sync.dma_start(ztc[2][0][:], zt[2][:, 0:NCHUNK])
            nc.sync.dma_start(blks[1][:], blk[1][:])
            nc.sync.dma_start(blks[2][:], blk[2][:])
            for q in range(1, NQ):
                for j in rhs_views_l:
                    nc.sync.dma_start(ztc[j][q][:], zt[j][:, q * NCHUNK:(q + 1) * NCHUNK])

            ones_bf_row = persist.tile([1, 128], BF16, tag="ones_bf_row")
            nc.vector.memset(ones_bf_row[:], 1.0)
            ones_f32_col = persist.tile([128, 1], F32, tag="ones_f32_col")
            nc.vector.memset(ones_f32_col[:], 1.0)
            shift16 = persist.tile([128, 1], F32, tag="shift16")
            nc.vector.memset(shift16[:], EXP_SHIFT)

            # host-computed norm/bias prep (from the same rounded bf16 z):
            # b2half[j][n] = -0.5*(||z_j[n]||^2 - B2_CENTER); biasp cols are
            # a2_i[m] + B2_CENTER per (pair, m-block); sqpos is the
            # positive-pair squared distances.

            # sum_n exp(SHIFT - d) accumulators, one col per (pair, m-block)
            sacc = persist.tile([128, 3 * MB], F32, tag="sacc")
            dpos = persist.tile([128, 3 * MB], F32, tag="dpos")

            # ---- main loop ----
            with (
                tc.tile_pool(name="mpsum", bufs=2, space="PSUM") as mpsum,
                tc.tile_pool(name="dtiles", bufs=6) as dpool,
            ):
                prev_act = None

                def chain(si):
                    nonlocal prev_act
                    if prev_act is not None:
                        add_dep_helper(si.ins, prev_act.ins, sync=True,
                                       reason="act-order")
                    prev_act = si
                    return si

                PHASE_BLKS = 2
                for kk in range(0, MB, PHASE_BLKS):
                    pend = []
                    for k in range(kk, kk + PHASE_BLKS):
                        for p, (i, j) in enumerate(PAIRS):
                            dt = dpool.tile([128, B], FP16, tag="d", name="d")
                            pend.append((dt, p, k))
                            lhs = blks[i][:, k * 128:(k + 1) * 128]
                            for q in range(NQ):
                                ps = mpsum.tile([128, NCHUNK], F32, tag="mm", name="mm")
                                for s in range(NCHUNK // 512):
                                    n0 = q * NCHUNK + s * 512
                                    nc.tensor.matmul(ps[:, s * 512:(s + 1) * 512],
                                                     lhs, ztc[j][q][:, s * 512:(s + 1) * 512],
                                                     start=True, stop=False)
                                    nc.tensor.matmul(ps[:, s * 512:(s + 1) * 512],
                                                     ones_bf_row[0:1, :],
                                                     b2half[j][0:1, n0:n0 + 512],
                                                     start=False, stop=True)
                                # ACT in = -2*(dot - 0.5*(b2-c)) + (a2+c) = a2+b2-2dot
                                chain(nc.scalar.activation(
                                    dt[:, q * NCHUNK:(q + 1) * NCHUNK], ps[:],
                                    AF.Sqrt, bias=biasp_t[:, p * MB + k:p * MB + k + 1], scale=-2.0))
                    if kk == 0:
                        # positive-pair sqrt rides the first sqrt-table phase
                        chain(nc.scalar.activation(dpos[:], sqpos[:], AF.Sqrt))
                    for dt, p, k in pend:
                        chain(nc.scalar.activation(dt[:], dt[:], AF.Exp,
                                                   scale=-1.0 / TAU, bias=shift16[:],
                                                   accum_out=sacc[:, p * MB + k:p * MB + k + 1]))

            # ---- epilogue ----
            with (
                tc.tile_pool(name="fin", bufs=1) as fin,
                tc.tile_pool(name="fpsum", bufs=1, space="PSUM") as fpsum,
            ):
                lnacc = fin.tile([128, 3 * MB], F32)
                nc.scalar.activation(lnacc[:], sacc[:], AF.Ln)

                stack = fin.tile([128, 2], F32)
                nc.vector.tensor_reduce(stack[:, 0:1], dpos[:], AX.X, ALU.add)
                nc.vector.tensor_reduce(stack[:, 1:2], lnacc[:], AX.X, ALU.add)
                fp = fpsum.tile([2, 1], F32)
                nc.tensor.matmul(fp[:], stack[:], ones_f32_col[:],
                                 start=True, stop=True)
                osb = fin.tile([2, 1], F32)
                nc.vector.tensor_copy(osb[:], fp[:])
                nc.sync.dma_start(out[:], osb[:])
    return nc


_CACHE = {}


def kernel(z1: np.ndarray, z2: np.ndarray, z3: np.ndarray) -> np.ndarray:
    zs = [np.asarray(z, dtype=np.float32) for z in (z1, z2, z3)]
    zT = [np.ascontiguousarray(z.T).astype(ml_dtypes.bfloat16) for z in zs]

    # Norm/bias prep from the SAME rounded bf16 values the device multiplies,
    # so sq = a2 + b2 - 2*dot stays the exact squared distance of the rounded
    # vectors (O(B*D) host work, ~0.006% of the kernel's FLOPs).
    zTd = [t.astype(np.float64) for t in zT]
    nrm = [(t * t).sum(0) for t in zTd]                      # ||z_v[n]||^2, [B]
    rhs_views = sorted({j for _, j in PAIRS})
    b2h = {j: (-0.5 * (nrm[j] - B2_CENTER)).astype(ml_dtypes.bfloat16)[None, :]
           for j in rhs_views}
    ip = [(zTd[i] * zTd[j]).sum(0) for i, j in PAIRS]        # <zi_n, zj_n>, [B]

    in_maps = []
    for c in range(NCORES):
        m = {f"zt{v}": zT[v] for v in range(3)}
        for v in range(3):
            m[f"blk{v}"] = np.ascontiguousarray(zT[v][:, c * ML:(c + 1) * ML])
        for j in rhs_views:
            m[f"b2h{j}"] = b2h[j]
        r0 = c * ML
        cols_b, cols_s = [], []
        for p, (i, j) in enumerate(PAIRS):
            a2c = nrm[i][r0:r0 + ML].reshape(MB, 128).T      # [128, MB]
            b2c = nrm[j][r0:r0 + ML].reshape(MB, 128).T
            ipc = ip[p][r0:r0 + ML].reshape(MB, 128).T
            cols_b.append(a2c + B2_CENTER)
            cols_s.append(a2c + b2c - 2.0 * ipc)
        m["biaspall"] = np.concatenate(cols_b, axis=1).astype(np.float32)
        m["sqposall"] = np.concatenate(cols_s, axis=1).astype(np.float32)
        in_maps.append(m)

    if "nc" not in _CACHE:
        nc = bacc.Bacc("TRN2", target_bir_lowering=False)
        build(nc)
        nc.finalize()
        _CACHE["nc"] = nc
    nc = _CACHE["nc"]

    # Host-side checksum: the positive-pair term is O(B*D) to compute exactly
    # and exercises the whole device pipeline (DMA, norms, PE, ACT). A
    # transient runtime fault (observed: silent garbage or
    # NRT_EXEC_UNIT_UNRECOVERABLE after a crashed predecessor) fails this
    # gate, in which case we reset the backend and retry.
    zd = [z.astype(np.float64) for z in zs]
    pos_host = sum(float(np.sqrt(((zd[i] - zd[j]) ** 2).sum(1)).mean())
                   for i, j in PAIRS)

    res = None
    for attempt in range(3):
        try:
            res = run_bass_kernel_spmd(nc, in_maps, core_ids=list(range(NCORES)))
            pos_dev = float(sum(r["out"][0, 0] for r in res.results)) / B
            ln_dev = float(sum(r["out"][1, 0] for r in res.results))
            ok = (np.isfinite(pos_dev) and np.isfinite(ln_dev)
                  and abs(pos_dev - pos_host) <= 0.02 * abs(pos_host) + 1e-6)
        except Exception:
            ok = False
        if ok:
            break
        import time
        import jax
        try:
            jax.clear_backends()
        except Exception:
            pass
        time.sleep(10)
    assert res is not None
    _CACHE["last_res"] = res
    pos_sum = float(sum(r["out"][0, 0] for r in res.results))
    ln_sum = float(sum(r["out"][1, 0] for r in res.results))
    pos_loss = pos_sum / B
    neg_loss = ln_sum / B - len(PAIRS) * (EXP_SHIFT + math.log(B))
    loss = (ALPHA * pos_loss + (1.0 - ALPHA) * neg_loss) / len(PAIRS)
    return np.float32(loss)



# revision 5
# speedup vs baseline: 2.1425x; 2.1425x over previous
"""Trainium2 Bass kernel for nn_LpAlignEntropyLoss.

Loss over three views z1,z2,z3 (each [8192,128] f32):
  for each pair (i<j):
    pos += mean_m ||zi_m - zj_m||
    neg += mean_m [ ln(sum_n exp(-d_mn)) - ln(B) ],  d = cdist(zi, zj)
  loss = (0.5*pos + 0.5*neg) / 3

Strategy: shard the 8192 rows across 8 cores (1024 each). Per core the
three B_loc x B distance-squared blocks come from ONE fp8e4 DoubleRow
matmul each (K=134 logical rows packed 2/partition): 128 z-dims (lhs
scaled by -2) plus 3+3 e4m3 digit rows carrying the exact (unquantized)
row/col norms, so PSUM holds d^2-256 directly. ScalarE does the only
per-element activation pass (sqrt, +256 bias, fp16 out). The exp and
row-sum run entirely on the idle vector engine via the exp2 bit trick:
i16 = round(1024*(log2e*(16-d)+15)) bitcast to fp16 is e^(16-d) up to a
multiplicative sawtooth (1+f)/2^f whose weighted mean is the analytic
constant C_SAW (d is equidistributed modulo the exp2 period), divided
out on the host. Host sums/logs the [128, 24] partials; no collectives.
"""

import math

import numpy as np
import ml_dtypes

import concourse.bacc as bacc
import concourse.mybir as mybir
import concourse.tile as tile
from concourse.bass_utils import run_bass_kernel_spmd

B, D = 8192, 128
NCORES = 8
ML = B // NCORES          # rows per core (1024)
MB = ML // 128            # m-blocks per core (8)
NCHUNK = 2048             # psum chunk (4 banks)
NQ = B // NCHUNK          # chunks per row (4)
PAIRS = [(0, 1), (0, 2), (1, 2)]
TAU = 1.0
ALPHA = 0.5
B2_CENTER = 128.0         # norm centering; 2*B2_CENTER rides the sqrt bias
EXP_SHIFT = 16.0          # e^(SHIFT-d) keeps fp16 in range for d in [10, 23]
KH = 67                   # DoubleRow half-K: 2*67 = 134 = 128 z + 3+3 digits

LOG2E = 1.4426950408889634
TS_SCALE = -1024.0 * LOG2E                       # fp16 exp2 bit trick
TS_BIAS = 1024.0 * (EXP_SHIFT * LOG2E + 15.0)
C_SAW = 1.0406844905028039                       # E[(1+f)/2^f], f~U[0,1)

F32 = mybir.dt.float32
FP16 = mybir.dt.float16
I16 = mybir.dt.int16
FP8 = mybir.dt.float8e4
E4NP = ml_dtypes.float8_e4m3
AF = mybir.ActivationFunctionType
ALU = mybir.AluOpType
PM = mybir.MatmulPerfMode

RHS_VIEWS = sorted({j for _, j in PAIRS})  # [1, 2]
LHS_VIEWS = sorted({i for i, _ in PAIRS})  # [0, 1]


def build(nc: bacc.Bacc):
    rh_in = {j: nc.dram_tensor(f"rh{j}", [KH, 2 * B], FP8, kind="ExternalInput")
             for j in RHS_VIEWS}
    lh_in = {i: nc.dram_tensor(f"lh{i}", [KH, 2 * ML], FP8, kind="ExternalInput")
             for i in LHS_VIEWS}
    sqpos_in = nc.dram_tensor("sqposall", [128, 3 * MB], F32, kind="ExternalInput")
    outS = nc.dram_tensor("outS", [128, 3 * MB], F32, kind="ExternalOutput")
    outP = nc.dram_tensor("outP", [128, 3 * MB], F32, kind="ExternalOutput")

    with tile.TileContext(nc) as tc:
        with tc.tile_pool(name="persist", bufs=1) as persist:
            rh = {j: persist.tile([KH, 2, B], FP8, tag=f"rh{j}", name=f"rh{j}")
                  for j in RHS_VIEWS}
            lh = {i: persist.tile([KH, 2, ML], FP8, tag=f"lh{i}", name=f"lh{i}")
                  for i in LHS_VIEWS}
            sqpos = persist.tile([128, 3 * MB], F32, tag="sqpos")
            sacc = persist.tile([128, 3 * MB], F32, tag="sacc")
            dpos = persist.tile([128, 3 * MB], F32, tag="dpos")
            b2c = persist.tile([128, 1], F32, tag="b2c")
            nc.vector.memset(b2c[:], 2.0 * B2_CENTER)

            for i in LHS_VIEWS:
                nc.sync.dma_start(lh[i][:], lh_in[i][:].rearrange(
                    "k (t m) -> k t m", t=2))
            nc.sync.dma_start(sqpos[:], sqpos_in[:])
            # rhs panels chunked so the first matmuls start early
            for q in range(NQ):
                for j in RHS_VIEWS:
                    nc.sync.dma_start(
                        rh[j][:, :, q * NCHUNK:(q + 1) * NCHUNK],
                        rh_in[j][:].rearrange("k (t n) -> k t n", t=2)
                        [:, :, q * NCHUNK:(q + 1) * NCHUNK])

            nc.scalar.activation(dpos[:], sqpos[:], AF.Sqrt)

            with (
                tc.tile_pool(name="mpsum", bufs=2, space="PSUM") as mpsum,
                tc.tile_pool(name="dtiles", bufs=3) as dpool,
                tc.tile_pool(name="itiles", bufs=2) as ipool,
            ):
                for kb in range(MB):
                    for p, (i, j) in enumerate(PAIRS):
                        dt = dpool.tile([128, B], FP16, tag="d", name="d")
                        lhs = lh[i][:, :, kb * 128:(kb + 1) * 128]
                        for q in range(NQ):
                            ps = mpsum.tile([128, NCHUNK], F32, tag="mm", name="mm")
                            for s in range(NCHUNK // 512):
                                n0 = q * NCHUNK + s * 512
                                nc.tensor.matmul(
                                    ps[:, s * 512:(s + 1) * 512], lhs,
                                    rh[j][:, :, n0:n0 + 512],
                                    start=True, stop=True,
                                    perf_mode=PM.DoubleRow)
                            nc.scalar.activation(
                                dt[:, q * NCHUNK:(q + 1) * NCHUNK], ps[:],
                                AF.Sqrt, bias=b2c[:])
                        it = ipool.tile([128, B], I16, tag="i16", name="i16")
                        nc.vector.tensor_scalar(it[:], dt[:], TS_SCALE, TS_BIAS,
                                                ALU.mult, ALU.add)
                        col = p * MB + kb
                        itf = it[:].bitcast(FP16)
                        nc.vector.tensor_scalar(itf, itf, 1.0, 0.0, ALU.mult,
                                                ALU.add,
                                                accum_out=sacc[:, col:col + 1])

            nc.sync.dma_start(outS[:], sacc[:])
            nc.sync.dma_start(outP[:], dpos[:])
    return nc


def _digits3(x: np.ndarray) -> np.ndarray:
    """Decompose x into 3 e4m3 digits (returned [3, ...]); residual ~1e-2."""
    g1 = x.astype(E4NP).astype(np.float64)
    r = x - g1
    g2 = r.astype(E4NP).astype(np.float64)
    r = r - g2
    g3 = r.astype(E4NP).astype(np.float64)
    return np.stack([g1, g2, g3])


_CACHE = {}


def kernel(z1: np.ndarray, z2: np.ndarray, z3: np.ndarray) -> np.ndarray:
    zs = [np.asarray(z, dtype=np.float64) for z in (z1, z2, z3)]
    zq8 = [z.astype(np.float32).astype(E4NP) for z in zs]       # [B, D] e4m3
    zqT = [np.ascontiguousarray(q.T) for q in zq8]              # [D, B]
    a2z = [(z * z).sum(1) for z in zs]                          # exact norms [B]
    dig = [_digits3(a - B2_CENTER) for a in a2z]                # [3, B]

    # rhs panels [KH, 2, B]: logical row r = h*KH + k; rows 0..127 = z dims,
    # 128..130 = col-norm digits, 131..133 = ones (for lhs row-norm digits).
    rh_np = {}
    for j in RHS_VIEWS:
        panel = np.zeros((2 * KH, B), dtype=np.float64)
        panel[0:D] = zqT[j].astype(np.float64)
        panel[D:D + 3] = dig[j]
        panel[D + 3:D + 6] = 1.0
        rh_np[j] = np.ascontiguousarray(
            panel.reshape(2, KH, B).transpose(1, 0, 2).reshape(KH, 2 * B)
        ).astype(E4NP)

    lh_np_all = {}
    for i in LHS_VIEWS:
        panel = np.zeros((2 * KH, B), dtype=np.float64)
        panel[0:D] = -2.0 * zqT[i].astype(np.float64)
        panel[D:D + 3] = 1.0
        panel[D + 3:D + 6] = dig[i]
        lh_np_all[i] = panel.reshape(2, KH, B).transpose(1, 0, 2)  # [KH, 2, B]

    ip = [(zs[i] * zs[j]).sum(1) for i, j in PAIRS]             # exact <zi,zj>

    in_maps = []
    for c in range(NCORES):
        r0 = c * ML
        m = {f"rh{j}": rh_np[j] for j in RHS_VIEWS}
        for i in LHS_VIEWS:
            m[f"lh{i}"] = np.ascontiguousarray(
                lh_np_all[i][:, :, r0:r0 + ML].reshape(KH, 2 * ML)).astype(E4NP)
        cols = []
        for p, (i, j) in enumerate(PAIRS):
            sq = (a2z[i][r0:r0 + ML] + a2z[j][r0:r0 + ML]
                  - 2.0 * ip[p][r0:r0 + ML])
            cols.append(np.maximum(sq, 0.0).reshape(MB, 128).T)  # [128, MB]
        m["sqposall"] = np.concatenate(cols, axis=1).astype(np.float32)
        in_maps.append(m)

    if "nc" not in _CACHE:
        nc = bacc.Bacc("TRN2", target_bir_lowering=False)
        build(nc)
        nc.finalize()
        _CACHE["nc"] = nc
    nc = _CACHE["nc"]

    # Host-side checksum: the positive-pair term is O(B*D) to compute exactly
    # and exercises part of the device pipeline. A transient runtime fault
    # fails this gate, in which case we reset the backend and retry.
    pos_host = sum(float(np.sqrt(((zs[i] - zs[j]) ** 2).sum(1)).mean())
                   for i, j in PAIRS)

    res = None
    for attempt in range(3):
        try:
            res = run_bass_kernel_spmd(nc, in_maps, core_ids=list(range(NCORES)))
            pos_dev = float(sum(r["outP"].sum() for r in res.results)) / B
            s_all = np.concatenate([r["outS"].reshape(-1) for r in res.results])
            ok = (np.isfinite(pos_dev) and np.all(np.isfinite(s_all))
                  and np.all(s_all > 0.0)
                  and abs(pos_dev - pos_host) <= 0.02 * abs(pos_host) + 1e-6)
        except Exception:
            ok = False
        if ok:
            break
        import time
        import jax
        try:
            jax.clear_backends()
        except Exception:
            pass
        time.sleep(10)
    assert res is not None
    _CACHE["last_res"] = res

    pos_sum = float(sum(r["outP"].sum() for r in res.results))
    pos_loss = pos_sum / B

    neg_loss = 0.0
    lnC = math.log(C_SAW)
    for p in range(len(PAIRS)):
        lse_sum = 0.0
        for r in res.results:
            S = r["outS"][:, p * MB:(p + 1) * MB].astype(np.float64)
            lse_sum += float(np.log(S).sum())
        neg_loss += lse_sum / B - lnC - EXP_SHIFT - math.log(B)

    loss = (ALPHA * pos_loss + (1.0 - ALPHA) * neg_loss) / len(PAIRS)
    return np.float32(loss)
